# revision 26
# baseline (speedup 1.0000x reference)
"""Trainium2 Bass kernel for nn_CFModel (KGAT-style GNN message passing).

Sharding: edges partitioned by dst-node range across 8 cores (aggregation is
core-local); node features replicated; one bf16 AllGather of h_nb between
layers.

Per core, per layer:
  - windows of 128 dst nodes (49/core). For each window and relation:
    V[p, r, :] = W_r @ tanh(W_r h_p + e_r)  via dense matmuls from resident
    transposed h (no dst-side gathers, relation_table folded in).
  - h_src rows for the edge slots are fetched per window with three 1024-row
    InstDMAGatherAnt gathers (the SWDGE descriptor ring caps one instruction
    at 1024 descriptors) at PAIR granularity: the source table is viewed as
    [25088, 256] bf16 row-pairs so indices fit int16; the wanted half is
    selected on DVE with a host-staged parity mask.  Layer 1 gathers from the
    host-staged padded entity table, layer 2 from the AllGather'd h_nb table.
  - edge tiles (128 edge slots, 24 tiles/window = 3 statically relation-pure
    tile slots per relation, SPMD-static graph): one-hot expansion matmuls
    pick v_e = V[dst_e, rel_e]; att_e = <h_src_e, v_e> (DVE); softmax without
    max-subtraction (logits bounded ~O(1)); binning matmuls accumulate
    [sum ex*h_src | sum ex] into window PSUM; h_nb = hU / max(s, eps).
  - All 49 V windows per layer are precomputed in a separate PSUM pool before
    the edge phase so the edge-phase pipeline runs with double-buffered PSUM.
Epilogue: out_i = lrelu((h+h_nb) W1_i^T) + lrelu((h*h_nb) W2_i^T), computed on
own rows; output is [h0 | out1 | out2] row-slice per core.
"""

import os
import sys

import numpy as np

sys.path.insert(0, "/opt/trn_rl_repo")

import ml_dtypes  # noqa: E402

from concourse import bacc, bass, mybir, tile  # noqa: E402
from concourse.bass_utils import run_bass_kernel_spmd  # noqa: E402

# ---------------- configuration ----------------
NCORES = 8
D = 128
R = 8
L = 2
NEG_SLOPE = 0.01

NPC = 6250          # real nodes per core
WINDOWS = 49        # 128-node windows per core
TPW = 24            # tiles per window
RELSLOT = TPW // R  # rel-pure tile slots
GROUP = 6           # tiles per group
NLOC = WINDOWS * 128
TILES = WINDOWS * TPW
GPW = TPW // GROUP
GROUPS = WINDOWS * GPW
NPAIR = NCORES * NLOC // 2          # pair rows in the gathered tables
GSPLIT = 3                          # gathers per window (1024 idxs each)
GTILES = TPW // GSPLIT              # tiles per gather
GIDX = GTILES * 128                 # idxs per gather (<= 1024 ring cap)
IDXCOL = TPW * 128 // 16            # int16 idx columns per window
NQ = 1                              # SWDGE queues for the gathers
assert TPW % GROUP == 0 and TPW % GSPLIT == 0 and GIDX <= 1024

BF16 = mybir.dt.bfloat16
F32 = mybir.dt.float32
I16 = mybir.dt.int16
I32 = mybir.dt.int32

PE = mybir.EngineType.PE
AF = mybir.ActivationFunctionType
OP = mybir.AluOpType

_CACHE = {}


def _build_nc():
    nc = bacc.Bacc("TRN2", target_bir_lowering=False, debug=False,
                   num_devices=NCORES, num_swdge_queues=NQ,
                   dynamic_dma_scratch_size=32768)

    dp = nc.declare_dram_parameter
    h0pair_ext = dp("h0pair", [NPAIR, 2 * D], BF16, isOutput=False)
    h0locT_ext = dp("h0locT", [D, NLOC], BF16, isOutput=False)
    h0loc_ext = dp("h0loc", [NLOC, D], F32, isOutput=False)
    gidx_ext = dp("gidx", [WINDOWS, 128, IDXCOL], I16, isOutput=False)
    meta_ext = dp("meta", [WINDOWS, 128, 2 * TPW], I32, isOutput=False)
    dstA_ext = dp("dstA", [WINDOWS, TPW * 128], BF16, isOutput=False)
    Wr_ext = dp("Wr", [R, D, D], BF16, isOutput=False)      # [r][d, f]
    WrT_ext = dp("WrT", [R, D, D], BF16, isOutput=False)    # [r][f, d]
    erow_ext = dp("erow", [R, D], BF16, isOutput=False)
    W1T_ext = dp("W1T", [L, D, D], BF16, isOutput=False)    # W1.T  [j, i]
    W2T_ext = dp("W2T", [L, D, D], BF16, isOutput=False)
    out_ext = dp("out", [NLOC, 3 * D], F32, isOutput=True)

    hnb1bf = nc.dram_tensor("hnb1bf", [NLOC, D], BF16)
    hnb2bf = nc.dram_tensor("hnb2bf", [NLOC, D], BF16)
    hnb_all = nc.dram_tensor("hnb_all", [NCORES * NLOC, D], BF16,
                             addr_space="Shared")
    hnb_loc = nc.dram_tensor("hnb_loc", [NCORES * NLOC, D], BF16)

    with tile.TileContext(nc) as tc:
        with (
            tc.tile_pool(name="const", bufs=1) as constp,
            tc.tile_pool(name="hTp", bufs=1) as hTp,
        ):
            # constants
            iota_i = constp.tile([128, 128], I32, tag="ioi")
            nc.gpsimd.iota(iota_i[:], pattern=[[1, 128]], base=0,
                           channel_multiplier=0)
            iota_bf = constp.tile([128, 128], BF16, tag="iobf")
            nc.vector.tensor_copy(out=iota_bf[:], in_=iota_i[:])
            iota_col_i = constp.tile([128, 1], I32, tag="ioci")
            nc.gpsimd.iota(iota_col_i[:], pattern=[[0, 1]], base=0,
                           channel_multiplier=1)
            iota_col = constp.tile([128, 1], F32, tag="ioc")
            nc.vector.tensor_copy(out=iota_col[:], in_=iota_col_i[:])
            ones_row = constp.tile([1, 128], BF16, tag="ones")
            nc.vector.memset(ones_row[:], 1.0)
            ones_col = constp.tile([128, 1], BF16, tag="onec")
            nc.vector.memset(ones_col[:], 1.0)

            Wr_sb = constp.tile([D, R, D], BF16, tag="wr")       # [d, r, f]
            nc.sync.dma_start(out=Wr_sb[:],
                              in_=Wr_ext[:, :, :].rearrange("r d f -> d r f"))
            WrT_sb = constp.tile([D, R, D], BF16, tag="wrt")     # [f, r, d]
            nc.sync.dma_start(out=WrT_sb[:],
                              in_=WrT_ext[:, :, :].rearrange("r f d -> f r d"))
            erow_sb = constp.tile([1, R, D], BF16, tag="er")
            nc.sync.dma_start(out=erow_sb[:], in_=erow_ext[None, :, :])
            W1T_sb = constp.tile([D, L, D], BF16, tag="w1t")     # [j, l, i]
            nc.sync.dma_start(out=W1T_sb[:],
                              in_=W1T_ext[:, :, :].rearrange("l j i -> j l i"))
            W2T_sb = constp.tile([D, L, D], BF16, tag="w2t")
            nc.sync.dma_start(out=W2T_sb[:],
                              in_=W2T_ext[:, :, :].rearrange("l j i -> j l i"))

            hT = hTp.tile([D, NLOC], BF16, tag="hT")

            Vp_cm = tc.tile_pool(name="Vp", bufs=WINDOWS)
            Vp = Vp_cm.__enter__()

            def window_phase(li):
                if li == 0:
                    nc.sync.dma_start(out=hT[:], in_=h0locT_ext[:, :])
                else:
                    nc.sync.dma_start_transpose(out=hT[:], in_=hnb1bf[:, :])
                V_ws = []
                with (
                    tc.tile_pool(name=f"win_ps{li}", bufs=2, space="PSUM") as win_ps,
                    tc.tile_pool(name=f"twtp{li}", bufs=2) as twtp,
                ):
                    for w in range(WINDOWS):
                        twt_ps = win_ps.tile([128, R * 128], F32, tag="wps")
                        for r in range(R):
                            sl = slice(r * 128, (r + 1) * 128)
                            nc.tensor.matmul(out=twt_ps[:, sl],
                                             lhsT=Wr_sb[:, r, :],
                                             rhs=hT[:, w * 128:(w + 1) * 128],
                                             start=True, stop=False)
                            nc.tensor.matmul(out=twt_ps[:, sl],
                                             lhsT=erow_sb[:, r, :],
                                             rhs=ones_row[:],
                                             start=False, stop=True)
                        twt_sb = twtp.tile([128, R * 128], BF16, tag="twt")
                        nc.scalar.activation(out=twt_sb[:], in_=twt_ps[:],
                                             func=AF.Tanh)
                        v_ps2 = win_ps.tile([128, R * 128], F32, tag="wps")
                        for r in range(R):
                            sl = slice(r * 128, (r + 1) * 128)
                            nc.tensor.matmul(out=v_ps2[:, sl],
                                             lhsT=twt_sb[:, sl],
                                             rhs=WrT_sb[:, r, :],
                                             start=True, stop=True)
                        V_w = Vp.tile([128, R * 128], BF16, tag="V")
                        nc.scalar.activation(out=V_w[:], in_=v_ps2[:],
                                             func=AF.Copy)
                        V_ws.append(V_w)
                return V_ws

            def edge_phase(li, V_ws):
                table = (h0pair_ext[:, :]
                         if li == 0 or os.environ.get("GNN_L2_FROM_INPUT") else
                         hnb_loc[:, :].rearrange("(a b) d -> a (b d)", b=2))
                with (
                    tc.tile_pool(name=f"idx{li}", bufs=3) as idxp,
                    tc.tile_pool(name=f"hsp{li}", bufs=2) as hsp,
                    tc.tile_pool(name=f"sel{li}", bufs=2) as selp,
                    tc.tile_pool(name=f"edge{li}", bufs=3) as edgep,
                    tc.tile_pool(name=f"prodp{li}", bufs=1) as prodp,
                    tc.tile_pool(name=f"bc_ps{li}", bufs=1, space="PSUM") as bc_psp,
                    tc.tile_pool(name=f"v_ps{li}", bufs=2, space="PSUM") as v_psp,
                    tc.tile_pool(name=f"hu_ps{li}", bufs=2, space="PSUM") as hu_psp,
                    tc.tile_pool(name=f"wout{li}", bufs=2) as outp,
                ):
                    for w in range(WINDOWS):
                        V_w = V_ws[w]
                        # ---- h_src staging: 3 pair-gathers + parity select
                        meta_t = idxp.tile([128, 2 * TPW], I32, tag="gi")
                        nc.sync.dma_start(out=meta_t[:], in_=meta_ext[w, :, :])
                        dstFw = meta_t[:, 0:TPW].bitcast(F32)
                        parw = meta_t[:, TPW:2 * TPW].bitcast(BF16)
                        idx_t = idxp.tile([128, IDXCOL], I16, tag="gx")
                        nc.sync.dma_start(out=idx_t[:], in_=gidx_ext[w, :, :])
                        dstAw = idxp.tile([1, TPW * 128], BF16, tag="daw")
                        nc.sync.dma_start(out=dstAw[:], in_=dstA_ext[w:w + 1, :])
                        hs_pair = hsp.tile([128, TPW, 2 * D], BF16, tag="hp")
                        for j in range(GSPLIT):
                            nc.gpsimd.dma_gather(
                                hs_pair[:, j * GTILES:(j + 1) * GTILES, :],
                                table,
                                idx_t[:, j * (GIDX // 16):(j + 1) * (GIDX // 16)],
                                GIDX, GIDX, 2 * D, queue_num=0)
                        pv = hs_pair[:].rearrange("p t (h d) -> p t h d", h=2)
                        even, dif = pv[:, :, 0, :], pv[:, :, 1, :]
                        # both tables are [even | odd-even]: hs = even + p*dif
                        # par is packed [p,p] bf16 so the broadcast keeps a
                        # packed last dim (2x DVE mode)
                        # hs2 = [selected h_src | 1]: the ones column
                        # rides the binning matmul as the softmax denominator
                        hs2_w = selp.tile([128, TPW, D + 1], BF16, tag="hs")
                        hs_w = hs2_w[:, :, 0:D]
                        par4 = parw.rearrange("p (t b) -> p t b", b=2)
                        nc.vector.tensor_tensor(
                            out=hs_w.rearrange("p t (a b) -> p t a b", b=2),
                            in0=dif.rearrange("p t (a b) -> p t a b", b=2),
                            in1=par4[:, :, None, :].to_broadcast(
                                [128, TPW, D // 2, 2]),
                            op=OP.mult)
                        nc.vector.tensor_tensor(out=hs_w, in0=hs_w,
                                                in1=even, op=OP.add)
                        nc.vector.memset(hs2_w[:, :, D:D + 1], 1.0)

                        hu_ps = hu_psp.tile([128, D + 1], F32, tag="hu")
                        for gw in range(GPW):
                            g = w * GPW + gw
                            t0 = w * TPW + gw * GROUP
                            FREE = GROUP * 128
                            hs_g = hs2_w[:, gw * GROUP:(gw + 1) * GROUP, 0:D]
                            dstrowA = dstAw[:, gw * FREE:(gw + 1) * FREE]
                            dstF_col = dstFw[:, gw * GROUP:(gw + 1) * GROUP]

                            bc_ps = bc_psp.tile([128, FREE], F32, tag="bc")
                            for a0 in range(0, FREE, 512):
                                a1 = min(a0 + 512, FREE)
                                nc.tensor.matmul(out=bc_ps[:, a0:a1],
                                                 lhsT=ones_row[:],
                                                 rhs=dstrowA[:, a0:a1],
                                                 start=True, stop=True)
                            bcA = edgep.tile([128, FREE], BF16, tag="bca")
                            nc.scalar.activation(out=bcA[:], in_=bc_ps[:],
                                                 func=AF.Copy)
                            otnA = edgep.tile([128, GROUP, 128], BF16, tag="otna")
                            nc.vector.tensor_scalar(
                                out=otnA[:], in0=bcA[:].rearrange(
                                    "p (t e) -> p t e", e=128),
                                scalar1=iota_col[:, :1], scalar2=None,
                                op0=OP.is_equal)
                            # expansion matmuls (rel static per tile slot)
                            v_ps = v_psp.tile([128, GROUP, 128], F32, tag="vps")
                            for j in range(GROUP):
                                rel_j = ((t0 + j) % TPW) // RELSLOT
                                nc.tensor.matmul(
                                    out=v_ps[:, j, :], lhsT=otnA[:, j, :],
                                    rhs=V_w[:, rel_j * 128:(rel_j + 1) * 128],
                                    start=True, stop=True)

                            # attention: att = rowsum(v * hs)
                            prod = prodp.tile([128, GROUP, D], BF16, tag="prod")
                            nc.vector.tensor_tensor(out=prod[:], in0=v_ps[:],
                                                    in1=hs_g, op=OP.mult)
                            att = edgep.tile([128, GROUP], F32, tag="att")
                            nc.vector.tensor_reduce(out=att[:], in_=prod[:],
                                                    axis=mybir.AxisListType.X,
                                                    op=OP.add)
                            ex = edgep.tile([128, GROUP], F32, tag="ex")
                            nc.scalar.activation(out=ex[:], in_=att[:],
                                                 func=AF.Exp)

                            # ex-scaled binning one-hot: (iota == off) * ex
                            ote = edgep.tile([128, GROUP, 128], BF16, tag="ote")
                            for j in range(GROUP):
                                nc.vector.tensor_scalar(
                                    out=ote[:, j, :], in0=iota_bf[:, :],
                                    scalar1=dstF_col[:, j:j + 1],
                                    scalar2=ex[:, j:j + 1],
                                    op0=OP.is_equal, op1=OP.mult)

                            for j in range(GROUP):
                                nc.tensor.matmul(
                                    out=hu_ps[:, :], lhsT=ote[:, j, :],
                                    rhs=hs2_w[:, gw * GROUP + j, :],
                                    start=(gw == 0 and j == 0),
                                    stop=(gw == GPW - 1 and j == GROUP - 1))

                        # window epilogue: h_nb = hU / max(s, eps)
                        hu_sb = outp.tile([128, D + 1], F32, tag="husb")
                        nc.scalar.activation(out=hu_sb[:], in_=hu_ps[:],
                                             func=AF.Copy)
                        s_cl = outp.tile([128, 1], F32, tag="scl")
                        nc.vector.tensor_scalar(out=s_cl[:],
                                                in0=hu_sb[:, D:D + 1],
                                                scalar1=1e-20, scalar2=None,
                                                op0=OP.max)
                        s_inv = outp.tile([128, 1], F32, tag="sinv")
                        nc.vector.reciprocal(out=s_inv[:], in_=s_cl[:])
                        hnb_bf_w = outp.tile([128, D], BF16, tag="hnbbf")
                        nc.scalar.activation(out=hnb_bf_w[:], in_=hu_sb[:, 0:D],
                                             func=AF.Copy, scale=s_inv[:, :1])
                        dst_bf = hnb1bf if li == 0 else hnb2bf
                        nc.sync.dma_start(out=dst_bf[w * 128:(w + 1) * 128, :],
                                          in_=hnb_bf_w[:])

            V1 = window_phase(0)
            if not os.environ.get("GNN_SKIP_L1E"):
                edge_phase(0, V1)
            if not os.environ.get("GNN_SKIP_COLL"):
                nc.gpsimd.collective_compute(
                    "AllGather", OP.bypass,
                    replica_groups=[list(range(NCORES))],
                    ins=[hnb1bf[:, :]], outs=[hnb_all[:, :]])
                # shared->local copy via SBUF, rewriting pair rows as
                # [even | odd-even] so the edge-phase select skips the sub
                pall = hnb_all[:, :].rearrange("(a b) d -> a (b d)", b=2)
                ploc = hnb_loc[:, :].rearrange("(a b) d -> a (b d)", b=2)
                CCH = 2048
                with tc.tile_pool(name="cpp", bufs=3) as cpp:
                    off = 0
                    while off < NPAIR:
                        n = min(CCH, NPAIR - off)
                        ct = cpp.tile([128, CCH // 128, 2 * D], BF16, tag="cp")
                        cv = ct[:, 0:n // 128, :]
                        nc.sync.dma_start(
                            out=cv,
                            in_=pall[off:off + n, :].rearrange(
                                "(a p) d -> p a d", p=128))
                        pvw = cv.rearrange("p a (h d) -> p a h d", h=2)
                        nc.vector.tensor_tensor(
                            out=pvw[:, :, 1, :], in0=pvw[:, :, 1, :],
                            in1=pvw[:, :, 0, :], op=OP.subtract)
                        nc.sync.dma_start(
                            out=ploc[off:off + n, :].rearrange(
                                "(a p) d -> p a d", p=128),
                            in_=cv)
                        off += n
            # L2 window phase after the collective issue (overlaps it)
            V2 = window_phase(1)
            if not os.environ.get("GNN_SKIP_L2"):
                edge_phase(1, V2)
            Vp_cm.__exit__(None, None, None)

            # ---------------- epilogue ----------------
            with (
                tc.tile_pool(name="ep", bufs=3) as ep,
                tc.tile_pool(name="epT", bufs=1) as epT,
                tc.tile_pool(name="ep_ps", bufs=2, space="PSUM") as ep_ps,
            ):
                # hT currently holds hnb1T (layer 2's input).  Build hnb2T.
                h0T = epT.tile([D, NLOC], BF16, tag="h0T")
                nc.sync.dma_start(out=h0T[:], in_=h0locT_ext[:, :])
                h2T = epT.tile([D, NLOC], BF16, tag="h2T")
                nc.sync.dma_start_transpose(out=h2T[:], in_=hnb2bf[:, :])

                aT = epT.tile([D, L, NLOC], BF16, tag="aT")
                nc.vector.tensor_tensor(out=aT[:, 0, :], in0=h0T[:], in1=hT[:],
                                        op=OP.add)
                nc.vector.tensor_tensor(out=aT[:, 1, :], in0=hT[:], in1=h2T[:],
                                        op=OP.add)
                mT = epT.tile([D, L, NLOC], BF16, tag="mT")
                nc.vector.tensor_tensor(out=mT[:, 0, :], in0=h0T[:], in1=hT[:],
                                        op=OP.mult)
                nc.vector.tensor_tensor(out=mT[:, 1, :], in0=hT[:], in1=h2T[:],
                                        op=OP.mult)

                for w in range(WINDOWS):
                    sl = slice(w * 128, (w + 1) * 128)
                    ow3 = ep.tile([128, 3 * D], F32, tag="ow3")
                    nc.sync.dma_start(out=ow3[:, 0:D], in_=h0loc_ext[sl, :])
                    for li in range(L):
                        ps1 = ep_ps.tile([128, D], F32, tag="ps1")
                        nc.tensor.matmul(out=ps1[:], lhsT=aT[:, li, sl],
                                         rhs=W1T_sb[:, li, :],
                                         start=True, stop=True)
                        ps2 = ep_ps.tile([128, D], F32, tag="ps2")
                        nc.tensor.matmul(out=ps2[:], lhsT=mT[:, li, sl],
                                         rhs=W2T_sb[:, li, :],
                                         start=True, stop=True)
                        lr1 = ep.tile([128, D], F32, tag="lr1")
                        nc.scalar.activation(out=lr1[:], in_=ps1[:],
                                             func=AF.Lrelu, alpha=NEG_SLOPE)
                        lr2 = ep.tile([128, D], F32, tag="lr2")
                        nc.scalar.activation(out=lr2[:], in_=ps2[:],
                                             func=AF.Lrelu, alpha=NEG_SLOPE)
                        nc.vector.tensor_tensor(
                            out=ow3[:, (1 + li) * D:(2 + li) * D],
                            in0=lr1[:], in1=lr2[:], op=OP.add)
                    nc.sync.dma_start(out=out_ext[sl, :], in_=ow3[:])

    nc.compile()
    return nc


# ---------------- host preprocessing ----------------

def _host_prep(node_ids, relation_ids, src, dst, entity_table, relation_table,
               relation_W, res_fc_W, res_fc2_W):
    node_ids = np.asarray(node_ids).astype(np.int64)
    rel = np.asarray(relation_ids).astype(np.int64)
    src = np.asarray(src).astype(np.int64)
    dst = np.asarray(dst).astype(np.int64)
    entity_table = np.asarray(entity_table, dtype=np.float32)
    relation_table = np.asarray(relation_table, dtype=np.float32)
    relation_W = np.asarray(relation_W, dtype=np.float32)
    res_fc_W = np.asarray(res_fc_W, dtype=np.float32)
    res_fc2_W = np.asarray(res_fc2_W, dtype=np.float32)

    bf = ml_dtypes.bfloat16
    in_maps = []
    # shared weight tensors
    Wr_bf = relation_W.astype(bf)                       # [r, d, f]
    WrT_bf = np.ascontiguousarray(relation_W.transpose(0, 2, 1)).astype(bf)
    erow_bf = relation_table.astype(bf)
    W1T_bf = np.ascontiguousarray(res_fc_W.transpose(0, 2, 1)).astype(bf)
    W2T_bf = np.ascontiguousarray(res_fc2_W.transpose(0, 2, 1)).astype(bf)

    # padded-numbered full h0 table (same for every core): row c*NLOC+j =
    # entity_table[node_ids[c*NPC+j]] for j < NPC, else 0
    h0full = np.zeros((NCORES * NLOC, D), np.float32)
    for c in range(NCORES):
        h0full[c * NLOC:c * NLOC + NPC] = entity_table[
            node_ids[c * NPC:(c + 1) * NPC]]
    # L1 pair table [even | odd-even] so the on-chip select skips the subtract
    h0p = h0full.reshape(NPAIR, 2, D).copy()
    h0p[:, 1, :] -= h0p[:, 0, :]
    h0pair = h0p.reshape(NPAIR, 2 * D).astype(bf)

    core_of = dst // NPC
    for c in range(NCORES):
        emask = core_of == c
        e_rel = rel[emask]
        e_src = src[emask]
        e_dst = dst[emask]
        loc = e_dst - c * NPC
        wl = loc // 128
        order = np.lexsort((e_dst, e_rel, wl))
        e_rel, e_src, loc, wl = (e_rel[order], e_src[order], loc[order],
                                 wl[order])

        slot_src = np.zeros(TILES * 128, np.int64)
        slot_off = np.full(TILES * 128, -1.0, np.float32)
        slot_rel = np.full(TILES * 128, -1, np.int64)
        for w in range(WINDOWS):
            m = wl == w
            w_src = e_src[m]
            w_off = (loc[m] - w * 128).astype(np.float32)
            w_rel = e_rel[m]
            base = w * TPW * 128
            # rel-pure static slots: rel r at tiles [r*RELSLOT, (r+1)*RELSLOT)
            for rv in range(R):
                rm = w_rel == rv
                cnt = int(rm.sum())
                cap = RELSLOT * 128
                assert cnt <= cap, f"rel slot overflow: {cnt} > {cap}"
                pos = base + rv * cap
                slot_src[pos:pos + cnt] = w_src[rm]
                slot_off[pos:pos + cnt] = w_off[rm]
                slot_rel[pos:pos + cnt] = rv

        offA = slot_off.reshape(TILES, 128)

        def to_g(x, width=None):
            # x: [TILES*128(, width)] slot-major -> [GROUPS, 128, GROUP(, w)]
            x = x.reshape(GROUPS, GROUP, 128, -1)
            return np.ascontiguousarray(x.transpose(0, 2, 1, 3))

        # gather indices: padded-table row per slot
        row = (slot_src // NPC) * NLOC + (slot_src % NPC)
        row[slot_rel < 0] = 0
        pairidx = (row >> 1).astype(np.int16)
        parity = (row & 1).astype(np.float32)
        parity[slot_rel < 0] = 0.0
        # gidx[w, p, j*64+c] = pairidx[w*3072 + j*1024 + c*16 + (p%16)]
        gi = pairidx.reshape(WINDOWS, GSPLIT, GIDX // 16, 16)
        gi = np.ascontiguousarray(gi.transpose(0, 3, 1, 2)).reshape(
            WINDOWS, 16, IDXCOL)
        gidx = np.tile(gi, (1, 8, 1))
        # meta[w, p, 0:T] = dst offset bits; [w, p, T:2T] = parity as
        # duplicated bf16 pairs packed into i32
        dstF = slot_off.reshape(WINDOWS, TPW, 128).transpose(0, 2, 1)
        par = parity.reshape(WINDOWS, TPW, 128).transpose(0, 2, 1)
        par2 = np.ascontiguousarray(
            np.repeat(par.astype(bf)[..., None], 2, axis=3))
        par2 = par2.reshape(WINDOWS, 128, 2 * TPW).view(np.int32)
        meta = np.ascontiguousarray(np.concatenate(
            [dstF.astype(np.float32).view(np.int32), par2], axis=2))
        dstA = offA.reshape(WINDOWS, TPW * 128).astype(bf)

        own = node_ids[c * NPC:(c + 1) * NPC]
        h0loc = np.zeros((NLOC, D), np.float32)
        h0loc[:NPC] = entity_table[own]
        h0locT = np.ascontiguousarray(h0loc.T).astype(bf)

        in_maps.append({
            "h0pair": h0pair, "h0locT": h0locT, "h0loc": h0loc,
            "gidx": gidx, "meta": meta, "dstA": dstA,
            "Wr": Wr_bf, "WrT": WrT_bf, "erow": erow_bf,
            "W1T": W1T_bf, "W2T": W2T_bf,
        })
    return in_maps


def kernel(**inputs):
    if "nc" not in _CACHE:
        _CACHE["nc"] = _build_nc()
    nc = _CACHE["nc"]
    in_maps = _host_prep(**inputs)
    res = run_bass_kernel_spmd(nc, in_maps, core_ids=list(range(NCORES)))
    out = np.concatenate([res.results[c]["out"][:NPC] for c in range(NCORES)],
                         axis=0)
    return out.astype(np.float32)
